# revision 4
# baseline (speedup 1.0000x reference)
"""Trainium2 Bass kernel for nn_CASCADES_v8_ResonantCore (moe_routing):

Computation (per batch b):
    centroid = 0.7*x[b,-1,:] + 0.3*mean_s(x[b])
    w = softmax(cos_sim(centroid, core_keys)/TEMP)      # [K]
    Lam = sum_k w[k] * core_pool[k]                     # [R,R]
    out[b] = ((x[b] @ V^T) @ Lam^T) @ U^T               # [S,D]

Strategy (8 cores, data-parallel over (batch, seq-half)):
  - Host: exact f64 routing; W_b = (U @ Lam_b)^T folded to one [R, D]
    weight per batch.  Output is written int8 with a per-column scale
    s_d = 8*sigma_d/127 (sigma_d^2 = W_d^T (V V^T) W_d) folded into
    the weight; host dequantizes.  f32->int8 on DVE/ACT rounds-to-
    nearest and saturates (HW-probed), so |err| <= s_d/2 ~ 0.03 sigma.
  - ALL DMA rides the single SP HWDGE ring, which executes FIFO:
    consts, then the whole 16.8 MiB x stream as 32x512 KiB chunks,
    then the int8 writes.  This gives reads strict priority (measured
    ~420 GB/s single-ring), so the tail mm2 never waits on a
    read/write-interleaved stream.
  - Device per seq-group of 512 rows: 32 accumulating [128,8]x[128,512]
    matmuls -> xv^T [8,512]; a one-matmul partition-replication with
    rp1 -> xvr [128,512]; 4x8 expansion matmuls; paired-bank drains
    ([128,1024] f32->int8 per copy, alternating DVE/ACT).  PE order is
    software-pipelined (mm1 of group g+1 before mm2 of group g).
  - HBM traffic per core: 16.8 MiB read + 8.4 MiB write ~= 25.3 MiB.
"""

import sys

sys.path.insert(0, "/opt/trn_rl_repo")

import contextlib

import ml_dtypes
import numpy as np

import concourse.bass as bass  # noqa: F401  (registers bass types)
import concourse.tile as tile
from concourse import bacc, mybir
from concourse.bass_utils import run_bass_kernel_spmd

BF16 = ml_dtypes.bfloat16

B, S, D, R, K = 4, 4096, 4096, 8, 4
NCORES = 8
SH = S // 2     # 2048 seq rows per core
G = 4           # seq groups per core
SG = SH // G    # 512 seq rows per group
NCH = D // 128  # 32 d-chunks
TPG = 2         # x tiles per group ([128, 8192] each)
CPT = NCH // TPG  # 16 d-chunks per x tile
NSUB = 4        # read sub-DMAs per tile (512 KiB each)
CPS = CPT // NSUB  # 4 d-chunks per sub-DMA
NSX = SG // 128   # 4 output strips per group
NDJ = D // 1024   # 4 paired-bank drain units per strip
EPS, TEMP = 1e-8, 0.05
QC = 8.0        # int8 scale: s_d = QC * sigma_d / 127

_cache = {}


def build_fused():
    """xtp [1024, 8192] bf16, vt [128, 256] bf16, wt [8, 4096] bf16
    (per-column-scaled W'), rp1/rp16 [8, 128] bf16 -> out [128, 65536]
    int8 with out[p, (g*4+q)*4096 + d] = out_rows[g*512 + q*128 + p, d]."""
    nc = bacc.Bacc("TRN2", target_bir_lowering=False, debug=False)
    xtp = nc.dram_tensor(
        "xtp", [G * TPG * 128, CPT * SG], mybir.dt.bfloat16, kind="ExternalInput"
    ).ap()
    vt = nc.dram_tensor("vt", [128, NCH * R], mybir.dt.bfloat16, kind="ExternalInput").ap()
    wt = nc.dram_tensor("wt", [R, D], mybir.dt.bfloat16, kind="ExternalInput").ap()
    rp1 = nc.dram_tensor("rp1", [R, 128], mybir.dt.bfloat16, kind="ExternalInput").ap()
    rp16 = nc.dram_tensor("rp16", [R, 128], mybir.dt.bfloat16, kind="ExternalInput").ap()
    out = nc.dram_tensor("out", [128, G * NSX * D], mybir.dt.int8, kind="ExternalOutput").ap()

    with tile.TileContext(nc) as tc:
        with contextlib.ExitStack() as ctx:
            cpool = ctx.enter_context(tc.tile_pool(name="consts", bufs=1))
            xpool = ctx.enter_context(tc.tile_pool(name="x", bufs=6))
            v8pool = ctx.enter_context(tc.tile_pool(name="xv8", bufs=2))
            vrpool = ctx.enter_context(tc.tile_pool(name="xvr", bufs=2))
            opool = ctx.enter_context(tc.tile_pool(name="ob", bufs=4))
            psA = ctx.enter_context(tc.tile_pool(name="psA", bufs=2, space="PSUM"))
            psR = ctx.enter_context(tc.tile_pool(name="psR", bufs=2, space="PSUM"))
            psB = ctx.enter_context(tc.tile_pool(name="psB", bufs=2, space="PSUM"))

            # consts head the FIFO ring; vt lands first for mm1(g0)
            vt_sb = cpool.tile([128, NCH * R], mybir.dt.bfloat16)
            nc.sync.dma_start(vt_sb[:], vt[:])
            wt_sb = cpool.tile([R, D], mybir.dt.bfloat16)
            nc.sync.dma_start(wt_sb[:], wt[:])
            rp1_sb = cpool.tile([R, 128], mybir.dt.bfloat16)
            nc.sync.dma_start(rp1_sb[:], rp1[:])
            rp16_sb = cpool.tile([R, 128], mybir.dt.bfloat16)
            nc.sync.dma_start(rp16_sb[:], rp16[:])
            wtr_sb = cpool.tile([128, D], mybir.dt.bfloat16)

            # the whole x stream, 512 KiB sub-DMAs, pushed ahead of all writes
            xs = []
            for t in range(G * TPG):
                xt = xpool.tile([128, CPT * SG], mybir.dt.bfloat16, tag="xs")
                for q in range(NSUB):
                    cols = slice(q * CPS * SG, (q + 1) * CPS * SG)
                    nc.sync.dma_start(xt[:, cols], xtp[t * 128:(t + 1) * 128, cols])
                xs.append(xt)

            def mm1(g):
                ps8 = psA.tile([R, SG], mybir.dt.float32, tag="ps8")
                for t2 in range(TPG):
                    for c in range(CPT):
                        ch = t2 * CPT + c
                        nc.tensor.matmul(
                            ps8[:],
                            vt_sb[:, ch * R:(ch + 1) * R],
                            xs[g * TPG + t2][:, c * SG:(c + 1) * SG],
                            start=(ch == 0),
                            stop=(ch == NCH - 1),
                        )
                xv8 = v8pool.tile([R, SG], mybir.dt.bfloat16, tag="xv8")
                nc.vector.tensor_copy(xv8[:], ps8[:])
                return xv8

            def repl(xv8):
                ps2 = psR.tile([128, SG], mybir.dt.float32, tag="ps2")
                nc.tensor.matmul(ps2[:], rp1_sb[:], xv8[:], start=True, stop=True)
                xvr = vrpool.tile([128, SG], mybir.dt.bfloat16, tag="xvr")
                nc.scalar.copy(xvr[:], ps2[:])
                return xvr

            def build_wtr():
                # wtr = wt[p%8]/16, built by 8 repmat matmuls, paired drains
                for u in range(D // 1024):
                    psw = psB.tile([128, 1024], mybir.dt.float32, tag="psb")
                    for h in range(2):
                        j = u * 2 + h
                        nc.tensor.matmul(
                            psw[:, h * 512:(h + 1) * 512], rp16_sb[:],
                            wt_sb[:, j * 512:(j + 1) * 512], start=True, stop=True,
                        )
                    dst = wtr_sb[:, u * 1024:(u + 1) * 1024]
                    if u % 2 == 0:
                        nc.vector.tensor_copy(dst, psw[:])
                    else:
                        nc.scalar.copy(dst, psw[:])

            def mm2(g, xvr, ob):
                # out strips [128, 4096] int8 = xv @ W'^T, paired-bank drains
                for i in range(NSX):
                    for u in range(NDJ):
                        ps = psB.tile([128, 1024], mybir.dt.float32, tag="psb")
                        for h in range(2):
                            j = u * 2 + h
                            nc.tensor.matmul(
                                ps[:, h * 512:(h + 1) * 512],
                                xvr[:, i * 128:(i + 1) * 128],
                                wtr_sb[:, j * 512:(j + 1) * 512],
                                start=True, stop=True,
                            )
                        dst = ob[:, i * D + u * 1024:i * D + (u + 1) * 1024]
                        if (i * NDJ + u) % 2 == 0:
                            nc.vector.tensor_copy(dst, ps[:])
                        else:
                            nc.scalar.copy(dst, ps[:])

            # software-pipelined schedule
            xv8_g = mm1(0)
            build_wtr()
            xvr_g = repl(xv8_g)
            obs = [opool.tile([128, NSX * D], mybir.dt.int8, tag="ob", name=f"ob{g}")
                   for g in range(G)]
            for g in range(1, G):
                xv8_n = mm1(g)
                mm2(g - 1, xvr_g, obs[g - 1])
                xvr_g = repl(xv8_n)
                off = (g - 1) * NSX * D
                nc.sync.dma_start(out[:, off:off + NSX * D], obs[g - 1][:])
            mm2(G - 1, xvr_g, obs[G - 1])
            for i in range(NSX):
                off = ((G - 1) * NSX + i) * D
                nc.sync.dma_start(out[:, off:off + D], obs[G - 1][:, i * D:(i + 1) * D])

    nc.compile()
    return nc


def _get_kernels():
    if "k" not in _cache:
        _cache["k"] = build_fused()
    return _cache["k"]


def _vt_layout(V, d, r):
    """[128, (d//128)*r] bf16 with vt[p, c*r + j] = V[j, c*128 + p]."""
    nch = d // 128
    return np.ascontiguousarray(
        V.reshape(r, nch, 128).transpose(2, 1, 0).reshape(128, nch * r)
    ).astype(BF16)


def _routing_weights(x, V_shared, U_shared, core_pool, core_keys):
    """Exact f64 routing on host -> per-batch (W'_b [R, D] bf16 scaled by
    1/s_d, s [B, D] f32 dequant scales)."""
    mean = x.mean(axis=1, dtype=np.float64)  # [B, D]
    centroid = 0.7 * x[:, -1, :].astype(np.float64) + 0.3 * mean
    c_n = centroid / np.maximum(
        np.linalg.norm(centroid, axis=-1, keepdims=True), EPS
    )
    kk = core_keys.astype(np.float64)
    k_n = kk / np.maximum(np.linalg.norm(kk, axis=-1, keepdims=True), EPS)
    sim = c_n @ k_n.T  # [B, K]
    logits = sim / TEMP
    e = np.exp(logits - logits.max(axis=-1, keepdims=True))
    w = e / e.sum(axis=-1, keepdims=True)
    Lam = np.einsum("bk,kij->bij", w, core_pool.astype(np.float64))  # [B, R, R]
    Wb = np.einsum("dr,brj->bjd", U_shared.astype(np.float64), Lam)  # [B, R, D]
    Vf = V_shared.astype(np.float64)
    C = Vf @ Vf.T  # [R, R]
    sig = np.sqrt(np.einsum("bjd,jk,bkd->bd", Wb, C, Wb))  # [B, D]
    s = (QC / 127.0) * np.maximum(sig, 1e-12)  # [B, D]
    wt_b = [np.ascontiguousarray(Wb[b] / s[b][None, :]).astype(BF16) for b in range(B)]
    return wt_b, s.astype(np.float32)


def _pack_xtp(xshard):
    """[SH, D] f32 -> [1024, 8192] bf16: tile t=g*2+t2 row p col c*SG+s
    = x[g*512 + s, (t2*16 + c)*128 + p]."""
    v = np.ascontiguousarray(
        xshard.reshape(G, SG, TPG, CPT, 128).transpose(0, 2, 4, 3, 1)
    )
    return v.reshape(G * TPG * 128, CPT * SG).astype(BF16)


def _rp_layout(r, scale):
    """[r, 128] bf16, rp[k, m] = (m % r == k)*scale: partition replicator."""
    m = np.arange(128)
    return ((m[None, :] % r == np.arange(r)[:, None]) * scale).astype(BF16)


def _shard_inputs(x, V_shared, U_shared, core_pool, core_keys):
    vt_np = _vt_layout(V_shared.astype(np.float32), D, R)
    rp1_np = _rp_layout(R, 1.0)
    rp16_np = _rp_layout(R, 1.0 / 16.0)
    wt_b, s = _routing_weights(x, V_shared, U_shared, core_pool, core_keys)
    in_maps = []
    for c in range(NCORES):
        b, h = c // 2, c % 2
        xtp_c = _pack_xtp(x[b, h * SH:(h + 1) * SH, :])
        in_maps.append({"xtp": xtp_c, "vt": vt_np, "wt": wt_b[b],
                        "rp1": rp1_np, "rp16": rp16_np})
    return in_maps, s


def kernel(x, V_shared, U_shared, core_pool, core_keys):
    x = np.asarray(x)
    V_shared = np.asarray(V_shared)
    U_shared = np.asarray(U_shared)
    core_pool = np.asarray(core_pool)
    core_keys = np.asarray(core_keys)

    nc = _get_kernels()
    core_ids = list(range(NCORES))
    in_maps, s = _shard_inputs(x, V_shared, U_shared, core_pool, core_keys)
    res = run_bass_kernel_spmd(nc, in_maps, core_ids).results

    out = np.empty((B, S, D), dtype=np.float32)
    for c in core_ids:
        b, h = c // 2, c % 2
        a = res[c]["out"].reshape(128, G, NSX, D).transpose(1, 2, 0, 3)
        out[b, h * SH:(h + 1) * SH, :] = (
            a.reshape(SH, D).astype(np.float32) * s[b][None, :]
        )
    return out


# revision 5
# speedup vs baseline: 1.0909x; 1.0909x over previous
"""Trainium2 Bass kernel for nn_CASCADES_v8_ResonantCore (moe_routing):

Computation (per batch b):
    centroid = 0.7*x[b,-1,:] + 0.3*mean_s(x[b])
    w = softmax(cos_sim(centroid, core_keys)/TEMP)      # [K]
    Lam = sum_k w[k] * core_pool[k]                     # [R,R]
    out[b] = ((x[b] @ V^T) @ Lam^T) @ U^T               # [S,D]

Strategy (8 cores, data-parallel over (batch, seq-half)):
  - Host: exact f64 routing; W_b = (U @ Lam_b)^T folded to one [R, D]
    weight per batch.  Output is written int8 with a per-column scale
    s_d = 8*sigma_d/127 (sigma_d^2 = W_d^T (V V^T) W_d) folded into
    the weight; host dequantizes.  f32->int8 on DVE/ACT rounds-to-
    nearest and saturates (HW-probed), so |err| <= s_d/2 ~ 0.03 sigma.
  - ALL DMA rides the single SP HWDGE ring, which executes FIFO:
    consts, then the whole 16.8 MiB x stream as 32x512 KiB chunks,
    then the int8 writes.  This gives reads strict priority (measured
    ~420 GB/s single-ring), so the tail mm2 never waits on a
    read/write-interleaved stream.
  - Device per seq-group of 512 rows: 32 accumulating [128,8]x[128,512]
    matmuls -> xv^T [8,512]; a one-matmul partition-replication with
    rp1 -> xvr [128,512]; 4x8 expansion matmuls; paired-bank drains
    ([128,1024] f32->int8 per copy, alternating DVE/ACT).  PE order is
    software-pipelined (mm1 of group g+1 before mm2 of group g).
  - HBM traffic per core: 16.8 MiB read + 8.4 MiB write ~= 25.3 MiB.
"""

import sys

sys.path.insert(0, "/opt/trn_rl_repo")

import contextlib

import ml_dtypes
import numpy as np

import concourse.bass as bass  # noqa: F401  (registers bass types)
import concourse.tile as tile
from concourse import bacc, mybir
from concourse.bass_utils import run_bass_kernel_spmd

BF16 = ml_dtypes.bfloat16

B, S, D, R, K = 4, 4096, 4096, 8, 4
NCORES = 8
SH = S // 2     # 2048 seq rows per core
G = 4           # seq groups per core
SG = SH // G    # 512 seq rows per group
NCH = D // 128  # 32 d-chunks
TPG = 2         # x tiles per group ([128, 8192] each)
CPT = NCH // TPG  # 16 d-chunks per x tile
NSUB = 4        # read sub-DMAs per tile (512 KiB each)
CPS = CPT // NSUB  # 4 d-chunks per sub-DMA
NSX = SG // 128   # 4 output strips per group
NDJ = D // 1024   # 4 paired-bank drain units per strip
EPS, TEMP = 1e-8, 0.05
QC = 8.0        # int8 scale: s_d = QC * sigma_d / 127

_cache = {}


def build_fused():
    """xtp [1024, 8192] bf16, vt [128, 256] bf16, wt [8, 4096] bf16
    (per-column-scaled W'), rp1/rp16 [8, 128] bf16 -> out [128, 65536]
    int8 with out[p, (g*4+q)*4096 + d] = out_rows[g*512 + q*128 + p, d]."""
    nc = bacc.Bacc("TRN2", target_bir_lowering=False, debug=False)
    xtp = nc.dram_tensor(
        "xtp", [G * TPG * 128, CPT * SG], mybir.dt.bfloat16, kind="ExternalInput"
    ).ap()
    vt = nc.dram_tensor("vt", [128, NCH * R], mybir.dt.bfloat16, kind="ExternalInput").ap()
    wt = nc.dram_tensor("wt", [R, D], mybir.dt.bfloat16, kind="ExternalInput").ap()
    rp1 = nc.dram_tensor("rp1", [R, 128], mybir.dt.bfloat16, kind="ExternalInput").ap()
    rp16 = nc.dram_tensor("rp16", [R, 128], mybir.dt.bfloat16, kind="ExternalInput").ap()
    out = nc.dram_tensor("out", [128, G * NSX * D], mybir.dt.int8, kind="ExternalOutput").ap()

    with tile.TileContext(nc) as tc:
        with contextlib.ExitStack() as ctx:
            cpool = ctx.enter_context(tc.tile_pool(name="consts", bufs=1))
            xpool = ctx.enter_context(tc.tile_pool(name="x", bufs=6))
            v8pool = ctx.enter_context(tc.tile_pool(name="xv8", bufs=2))
            vrpool = ctx.enter_context(tc.tile_pool(name="xvr", bufs=2))
            opool = ctx.enter_context(tc.tile_pool(name="ob", bufs=4))
            psA = ctx.enter_context(tc.tile_pool(name="psA", bufs=1, space="PSUM"))
            psR = ctx.enter_context(tc.tile_pool(name="psR", bufs=1, space="PSUM"))
            psB = ctx.enter_context(tc.tile_pool(name="psB", bufs=3, space="PSUM"))

            # consts head the FIFO ring; vt lands first for mm1(g0)
            vt_sb = cpool.tile([128, NCH * R], mybir.dt.bfloat16)
            nc.sync.dma_start(vt_sb[:], vt[:])
            wt_sb = cpool.tile([R, D], mybir.dt.bfloat16)
            nc.sync.dma_start(wt_sb[:], wt[:])
            rp1_sb = cpool.tile([R, 128], mybir.dt.bfloat16)
            nc.sync.dma_start(rp1_sb[:], rp1[:])
            rp16_sb = cpool.tile([R, 128], mybir.dt.bfloat16)
            nc.sync.dma_start(rp16_sb[:], rp16[:])
            wtr_sb = cpool.tile([128, D], mybir.dt.bfloat16)

            # the whole x stream, 512 KiB sub-DMAs, pushed ahead of all writes
            xs = []
            for t in range(G * TPG):
                xt = xpool.tile([128, CPT * SG], mybir.dt.bfloat16, tag="xs")
                for q in range(NSUB):
                    cols = slice(q * CPS * SG, (q + 1) * CPS * SG)
                    nc.sync.dma_start(xt[:, cols], xtp[t * 128:(t + 1) * 128, cols])
                xs.append(xt)

            def mm1(g):
                ps8 = psA.tile([R, SG], mybir.dt.float32, tag="ps8")
                for t2 in range(TPG):
                    for c in range(CPT):
                        ch = t2 * CPT + c
                        nc.tensor.matmul(
                            ps8[:],
                            vt_sb[:, ch * R:(ch + 1) * R],
                            xs[g * TPG + t2][:, c * SG:(c + 1) * SG],
                            start=(ch == 0),
                            stop=(ch == NCH - 1),
                        )
                xv8 = v8pool.tile([R, SG], mybir.dt.bfloat16, tag="xv8")
                nc.vector.tensor_copy(xv8[:], ps8[:])
                return xv8

            def repl(xv8):
                ps2 = psR.tile([128, SG], mybir.dt.float32, tag="ps2")
                nc.tensor.matmul(ps2[:], rp1_sb[:], xv8[:], start=True, stop=True)
                xvr = vrpool.tile([128, SG], mybir.dt.bfloat16, tag="xvr")
                nc.scalar.copy(xvr[:], ps2[:])
                return xvr

            def build_wtr():
                # wtr = wt[p%8]/16, built by 8 repmat matmuls, paired drains
                for u in range(D // 1024):
                    psw = psB.tile([128, 1024], mybir.dt.float32, tag="psb")
                    for h in range(2):
                        j = u * 2 + h
                        nc.tensor.matmul(
                            psw[:, h * 512:(h + 1) * 512], rp16_sb[:],
                            wt_sb[:, j * 512:(j + 1) * 512], start=True, stop=True,
                        )
                    dst = wtr_sb[:, u * 1024:(u + 1) * 1024]
                    if u % 2 == 0:
                        nc.vector.tensor_copy(dst, psw[:])
                    else:
                        nc.scalar.copy(dst, psw[:])

            def mm2(g, xvr, ob):
                # out strips [128, 4096] int8 = xv @ W'^T, paired-bank drains
                for i in range(NSX):
                    for u in range(NDJ):
                        ps = psB.tile([128, 1024], mybir.dt.float32, tag="psb")
                        for h in range(2):
                            j = u * 2 + h
                            nc.tensor.matmul(
                                ps[:, h * 512:(h + 1) * 512],
                                xvr[:, i * 128:(i + 1) * 128],
                                wtr_sb[:, j * 512:(j + 1) * 512],
                                start=True, stop=True,
                            )
                        dst = ob[:, i * D + u * 1024:i * D + (u + 1) * 1024]
                        if (i * NDJ + u) % 2 == 0:
                            nc.vector.tensor_copy(dst, ps[:])
                        else:
                            nc.scalar.copy(dst, ps[:])

            # software-pipelined schedule
            xv8_g = mm1(0)
            build_wtr()
            xvr_g = repl(xv8_g)
            obs = [opool.tile([128, NSX * D], mybir.dt.int8, tag="ob", name=f"ob{g}")
                   for g in range(G)]
            for g in range(1, G):
                xv8_n = mm1(g)
                mm2(g - 1, xvr_g, obs[g - 1])
                xvr_g = repl(xv8_n)
                off = (g - 1) * NSX * D
                nc.sync.dma_start(out[:, off:off + NSX * D], obs[g - 1][:])
            mm2(G - 1, xvr_g, obs[G - 1])
            for i in range(NSX):
                off = ((G - 1) * NSX + i) * D
                nc.sync.dma_start(out[:, off:off + D], obs[G - 1][:, i * D:(i + 1) * D])

    nc.compile()
    return nc


def _get_kernels():
    if "k" not in _cache:
        _cache["k"] = build_fused()
    return _cache["k"]


def _vt_layout(V, d, r):
    """[128, (d//128)*r] bf16 with vt[p, c*r + j] = V[j, c*128 + p]."""
    nch = d // 128
    return np.ascontiguousarray(
        V.reshape(r, nch, 128).transpose(2, 1, 0).reshape(128, nch * r)
    ).astype(BF16)


def _routing_weights(x, V_shared, U_shared, core_pool, core_keys):
    """Exact f64 routing on host -> per-batch (W'_b [R, D] bf16 scaled by
    1/s_d, s [B, D] f32 dequant scales)."""
    mean = x.mean(axis=1, dtype=np.float64)  # [B, D]
    centroid = 0.7 * x[:, -1, :].astype(np.float64) + 0.3 * mean
    c_n = centroid / np.maximum(
        np.linalg.norm(centroid, axis=-1, keepdims=True), EPS
    )
    kk = core_keys.astype(np.float64)
    k_n = kk / np.maximum(np.linalg.norm(kk, axis=-1, keepdims=True), EPS)
    sim = c_n @ k_n.T  # [B, K]
    logits = sim / TEMP
    e = np.exp(logits - logits.max(axis=-1, keepdims=True))
    w = e / e.sum(axis=-1, keepdims=True)
    Lam = np.einsum("bk,kij->bij", w, core_pool.astype(np.float64))  # [B, R, R]
    Wb = np.einsum("dr,brj->bjd", U_shared.astype(np.float64), Lam)  # [B, R, D]
    Vf = V_shared.astype(np.float64)
    C = Vf @ Vf.T  # [R, R]
    sig = np.sqrt(np.einsum("bjd,jk,bkd->bd", Wb, C, Wb))  # [B, D]
    s = (QC / 127.0) * np.maximum(sig, 1e-12)  # [B, D]
    wt_b = [np.ascontiguousarray(Wb[b] / s[b][None, :]).astype(BF16) for b in range(B)]
    return wt_b, s.astype(np.float32)


def _pack_xtp(xshard):
    """[SH, D] f32 -> [1024, 8192] bf16: tile t=g*2+t2 row p col c*SG+s
    = x[g*512 + s, (t2*16 + c)*128 + p]."""
    v = np.ascontiguousarray(
        xshard.reshape(G, SG, TPG, CPT, 128).transpose(0, 2, 4, 3, 1)
    )
    return v.reshape(G * TPG * 128, CPT * SG).astype(BF16)


def _rp_layout(r, scale):
    """[r, 128] bf16, rp[k, m] = (m % r == k)*scale: partition replicator."""
    m = np.arange(128)
    return ((m[None, :] % r == np.arange(r)[:, None]) * scale).astype(BF16)


def _shard_inputs(x, V_shared, U_shared, core_pool, core_keys):
    vt_np = _vt_layout(V_shared.astype(np.float32), D, R)
    rp1_np = _rp_layout(R, 1.0)
    rp16_np = _rp_layout(R, 1.0 / 16.0)
    wt_b, s = _routing_weights(x, V_shared, U_shared, core_pool, core_keys)
    in_maps = []
    for c in range(NCORES):
        b, h = c // 2, c % 2
        xtp_c = _pack_xtp(x[b, h * SH:(h + 1) * SH, :])
        in_maps.append({"xtp": xtp_c, "vt": vt_np, "wt": wt_b[b],
                        "rp1": rp1_np, "rp16": rp16_np})
    return in_maps, s


def kernel(x, V_shared, U_shared, core_pool, core_keys):
    x = np.asarray(x)
    V_shared = np.asarray(V_shared)
    U_shared = np.asarray(U_shared)
    core_pool = np.asarray(core_pool)
    core_keys = np.asarray(core_keys)

    nc = _get_kernels()
    core_ids = list(range(NCORES))
    in_maps, s = _shard_inputs(x, V_shared, U_shared, core_pool, core_keys)
    res = run_bass_kernel_spmd(nc, in_maps, core_ids).results

    out = np.empty((B, S, D), dtype=np.float32)
    for c in core_ids:
        b, h = c // 2, c % 2
        a = res[c]["out"].reshape(128, G, NSX, D).transpose(1, 2, 0, 3)
        out[b, h * SH:(h + 1) * SH, :] = (
            a.reshape(SH, D).astype(np.float32) * s[b][None, :]
        )
    return out
